# revision 9
# baseline (speedup 1.0000x reference)
"""Trainium2 Bass kernel for 2-layer GCN (nn_BasicGNN).

Strategy (8 NeuronCores, SPMD):
  - Reorder layer 2 as (A_norm @ z1) @ W2 so both aggregations move 16 feats.
  - out[v] = dinv[v] * (sum_{u->v} dinv[u]*h[u] + dinv[v]*h[v])  (self-loop)
    => per-node pre-scale by dinv, aggregate raw sums, post-scale by dinv.
  - Shard destinations across 8 cores (12500 each). Host sorts each core's
    dests by in-degree, pads edge lists per 128-dest group to the group max
    K_g, and maps edge sources to "table rows" (per-core slabs concatenated
    by AllGather).
  - Device per layer: gather [128, K_g*16] rows per group with one
    indirect DMA, strided reduce_sum on DVE, add self term, scale, next.
  - Weights (128x16, 16x40) replicated; AllGather shares the 16-feature
    tables between layers.
"""

import sys
import numpy as np

if "/opt/trn_rl_repo" not in sys.path:
    sys.path.insert(0, "/opt/trn_rl_repo")

N_CORES = 8
P = 128


def _preprocess(x, edge_index, W1, b1, W2, b2):
    x = np.asarray(x, dtype=np.float32)
    W1 = np.asarray(W1, dtype=np.float32)
    b1 = np.asarray(b1, dtype=np.float32)
    W2 = np.asarray(W2, dtype=np.float32)
    b2 = np.asarray(b2, dtype=np.float32)
    N, F_IN = x.shape
    F_HID = W1.shape[1]
    F_OUT = W2.shape[1]
    M = N_CORES
    assert N % M == 0
    Ns = N // M
    NsP = ((Ns + P - 1) // P) * P      # padded dest count per core
    G = NsP // P                        # dest groups per core
    S = NsP + P                         # slab rows (last P rows = zeros)
    PAD_ROW = NsP                       # table row that is guaranteed zero (core 0 zero block)

    row = np.asarray(edge_index[0]).astype(np.int64)
    col = np.asarray(edge_index[1]).astype(np.int64)
    deg = np.bincount(col, minlength=N).astype(np.int64) + 1
    dinv = (deg.astype(np.float64) ** -0.5).astype(np.float32)

    # Per-core degree-sorted dest permutation; node -> global table row
    pos_global = np.empty(N, dtype=np.int64)
    pos_of_list = []
    sorted_indeg = []
    for m in range(M):
        indeg = deg[m * Ns:(m + 1) * Ns] - 1
        key = np.concatenate([indeg, np.full(NsP - Ns, -1, dtype=np.int64)])
        order = np.argsort(key, kind="stable")
        pos_of = np.empty(NsP, dtype=np.int64)
        pos_of[order] = np.arange(NsP)
        pos_of_list.append(pos_of)
        sorted_indeg.append(np.maximum(key[order], 0))
        pos_global[m * Ns:(m + 1) * Ns] = m * S + pos_of[:Ns]

    # Shared per-group K (max over cores, >=1)
    Ks = np.zeros(G, dtype=np.int64)
    for m in range(M):
        si = sorted_indeg[m].reshape(G, P)
        Ks = np.maximum(Ks, si.max(axis=1))
    Ks = np.maximum(Ks, 1)
    offs = np.zeros(G + 1, dtype=np.int64)
    offs[1:] = np.cumsum(Ks)
    SUMK = int(offs[-1])

    in_maps = []
    for m in range(M):
        pos_of = pos_of_list[m]
        mask = (col >= m * Ns) & (col < (m + 1) * Ns)
        er = row[mask]
        dpos = pos_of[col[mask] - m * Ns]
        src_row = pos_global[er]
        o = np.argsort(dpos, kind="stable")
        dpos = dpos[o]
        src_row = src_row[o]
        cnt = np.bincount(dpos, minlength=NsP)
        starts = np.concatenate([[0], np.cumsum(cnt)])[:-1]
        rank = np.arange(len(dpos)) - starts[dpos]
        idx_all = np.full((P, SUMK), PAD_ROW, dtype=np.int32)
        g_of = dpos // P
        p_of = dpos % P
        idx_all[p_of, offs[g_of] + rank] = src_row.astype(np.int32)

        xp = np.zeros((NsP, F_IN), np.float32)
        xp[pos_of[:Ns]] = x[m * Ns:(m + 1) * Ns]
        xT = np.ascontiguousarray(xp.T)

        d_sorted = np.ones(NsP, np.float32)
        d_sorted[pos_of[:Ns]] = dinv[m * Ns:(m + 1) * Ns]
        dinv_col = np.ascontiguousarray(d_sorted.reshape(G, P).T)
        dinv2_col = np.ascontiguousarray(dinv_col * dinv_col)
        db1 = (d_sorted.reshape(G, P)[:, :, None] * b1[None, None, :])
        db1 = np.ascontiguousarray(db1.transpose(1, 0, 2).reshape(P, G * F_HID))

        in_maps.append({
            "xT": xT,
            "idx": idx_all,
            "dinv": dinv_col,
            "dinv2": dinv2_col,
            "db1": db1.astype(np.float32),
            "W1": W1,
            "W2": W2,
            "b2r": np.ascontiguousarray(np.tile(b2[None, :], (P, 1))),
        })

    meta = dict(N=N, Ns=Ns, NsP=NsP, G=G, S=S, Ks=Ks.tolist(), offs=offs.tolist(),
                SUMK=SUMK, F_IN=F_IN, F_HID=F_HID, F_OUT=F_OUT,
                pos_of_list=pos_of_list, b1_zero=bool(not np.any(b1)))
    return meta, in_maps


def _build_program(meta, dbg=False):
    import concourse.bacc as bacc
    import concourse.tile as tile
    import concourse.bass as bass
    import concourse.mybir as mybir
    from concourse.masks import make_identity

    f32 = mybir.dt.float32
    i32 = mybir.dt.int32
    G, S, NsP, SUMK = meta["G"], meta["S"], meta["NsP"], meta["SUMK"]
    Ks, offs = meta["Ks"], meta["offs"]
    F_IN, F_HID, F_OUT = meta["F_IN"], meta["F_HID"], meta["F_OUT"]
    M = N_CORES

    nc = bacc.Bacc("TRN2", target_bir_lowering=False, debug=False,
                   enable_asserts=False, num_devices=M)

    xT_d = nc.dram_tensor("xT", [P, NsP], f32, kind="ExternalInput")
    idx_d = nc.dram_tensor("idx", [P, SUMK], i32, kind="ExternalInput")
    dinv_d = nc.dram_tensor("dinv", [P, G], f32, kind="ExternalInput")
    dinv2_d = nc.dram_tensor("dinv2", [P, G], f32, kind="ExternalInput")
    db1_d = nc.dram_tensor("db1", [P, G * F_HID], f32, kind="ExternalInput")
    W1_d = nc.dram_tensor("W1", [F_IN, F_HID], f32, kind="ExternalInput")
    W2_d = nc.dram_tensor("W2", [F_HID, F_OUT], f32, kind="ExternalInput")
    b2r_d = nc.dram_tensor("b2r", [P, F_OUT], f32, kind="ExternalInput")
    out_d = nc.dram_tensor("out", [NsP, F_OUT], f32, kind="ExternalOutput")

    if dbg:
        dslab_d = nc.dram_tensor("dslab", [NsP, F_HID], f32, kind="ExternalOutput")
        dtab_d = nc.dram_tensor("dtab", [2048, F_HID], f32, kind="ExternalOutput")
        dagg_d = nc.dram_tensor("dagg", [NsP, F_HID], f32, kind="ExternalOutput")
    slab1 = nc.dram_tensor("slab1", [S, F_HID], f32, kind="Internal")
    slab2 = nc.dram_tensor("slab2", [S, F_HID], f32, kind="Internal")
    tab1 = nc.dram_tensor("tab1", [M * S, F_HID], f32, kind="Internal",
                          addr_space="Shared")
    tab2 = nc.dram_tensor("tab2", [M * S, F_HID], f32, kind="Internal",
                          addr_space="Shared")
    RG = [list(range(M))]

    with tile.TileContext(nc) as tc:
        with tc.tile_pool(name="big", bufs=1) as bigp, \
             tc.tile_pool(name="wts", bufs=1) as wp, \
             tc.tile_pool(name="work", bufs=8) as sb, \
             tc.tile_pool(name="gath", bufs=6) as gp, \
             tc.tile_pool(name="ps", bufs=2, space="PSUM") as pp:

            xT_s = bigp.tile([P, NsP], f32)
            nc.sync.dma_start(xT_s[:], xT_d[:])
            idx_s = bigp.tile([P, SUMK], i32)
            nc.sync.dma_start(idx_s[:], idx_d[:])
            dinv_s = wp.tile([P, G], f32)
            nc.sync.dma_start(dinv_s[:], dinv_d[:])
            dinv2_s = wp.tile([P, G], f32)
            nc.sync.dma_start(dinv2_s[:], dinv2_d[:])
            db1_s = wp.tile([P, G * F_HID], f32)
            nc.sync.dma_start(db1_s[:], db1_d[:])
            W1_s = wp.tile([F_IN, F_HID], f32)
            nc.sync.dma_start(W1_s[:], W1_d[:])
            W2_s = wp.tile([F_HID, F_OUT], f32)
            nc.sync.dma_start(W2_s[:], W2_d[:])
            b2r_s = wp.tile([P, F_OUT], f32)
            nc.sync.dma_start(b2r_s[:], b2r_d[:])
            ident = wp.tile([P, P], f32)
            make_identity(nc, ident[:])
            zt = wp.tile([P, F_HID], f32)
            nc.vector.memset(zt[:], 0.0)
            nc.sync.dma_start(slab1[NsP:NsP + P, :], zt[:])
            nc.sync.dma_start(slab2[NsP:NsP + P, :], zt[:])

            # ---- Phase A: gs1 = dinv * (x @ W1), write slab1 ----
            for g in range(G):
                g1p = pp.tile([P, F_HID], f32, tag="mm1")
                nc.tensor.matmul(g1p[:], lhsT=xT_s[:, g * P:(g + 1) * P],
                                 rhs=W1_s[:], start=True, stop=True)
                gs1 = sb.tile([P, F_HID], f32, tag="gs1")
                nc.vector.tensor_scalar_mul(gs1[:], g1p[:], dinv_s[:, g:g + 1])
                nc.sync.dma_start(slab1[g * P:(g + 1) * P, :], gs1[:])
                if dbg:
                    nc.sync.dma_start(dslab_d[g * P:(g + 1) * P, :], gs1[:])

            nc.gpsimd.collective_compute(
                "AllGather", mybir.AluOpType.bypass, replica_groups=RG,
                ins=[slab1[:]], outs=[tab1[:]])

            if dbg:
                for j in range(16):
                    half = 0 if j < 8 else 1
                    src0 = (j % 8) * P if half == 0 else S + (j % 8) * P
                    dt_t = sb.tile([P, F_HID], f32, tag="dtab")
                    nc.sync.dma_start(dt_t[:], tab1[src0:src0 + P, :])
                    nc.sync.dma_start(dtab_d[j * P:(j + 1) * P, :], dt_t[:])

            # ---- Phase B: s1 = gather-sum + self; gs2 = relu(dinv2*s1 + dinv*b1) ----
            for g in range(G):
                K = Ks[g]
                o = offs[g]
                gt = gp.tile([P, K, F_HID], f32, tag="gath")
                for k in range(K):
                    nc.gpsimd.indirect_dma_start(
                        out=gt[:, k, :], out_offset=None, in_=tab1[:],
                        in_offset=bass.IndirectOffsetOnAxis(
                            ap=idx_s[:, o + k:o + k + 1], axis=0))
                s1 = sb.tile([P, F_HID], f32, tag="s1")
                nc.vector.reduce_sum(out=s1[:], in_=gt[:].rearrange("p k f -> p f k"),
                                     axis=mybir.AxisListType.X)
                sf = sb.tile([P, F_HID], f32, tag="sf")
                nc.sync.dma_start(sf[:], slab1[g * P:(g + 1) * P, :])
                nc.vector.tensor_add(s1[:], s1[:], sf[:])
                if dbg:
                    nc.sync.dma_start(dagg_d[g * P:(g + 1) * P, :], s1[:])
                gs2 = sb.tile([P, F_HID], f32, tag="gs2")
                if meta.get("b1_zero"):
                    nc.vector.tensor_scalar(
                        out=gs2[:], in0=s1[:], scalar1=dinv2_s[:, g:g + 1],
                        scalar2=0.0, op0=mybir.AluOpType.mult,
                        op1=mybir.AluOpType.max)
                else:
                    nc.vector.tensor_scalar_mul(s1[:], s1[:], dinv2_s[:, g:g + 1])
                    nc.vector.tensor_add(s1[:], s1[:],
                                         db1_s[:, g * F_HID:(g + 1) * F_HID])
                    nc.vector.tensor_scalar_max(gs2[:], s1[:], 0.0)
                nc.sync.dma_start(slab2[g * P:(g + 1) * P, :], gs2[:])

            nc.gpsimd.collective_compute(
                "AllGather", mybir.AluOpType.bypass, replica_groups=RG,
                ins=[slab2[:]], outs=[tab2[:]])

            # ---- Phase C: s2 = gather-sum + self; out = (dinv*s2) @ W2 + b2 ----
            for g in range(G):
                K = Ks[g]
                o = offs[g]
                gt = gp.tile([P, K, F_HID], f32, tag="gath")
                for k in range(K):
                    nc.gpsimd.indirect_dma_start(
                        out=gt[:, k, :], out_offset=None, in_=tab2[:],
                        in_offset=bass.IndirectOffsetOnAxis(
                            ap=idx_s[:, o + k:o + k + 1], axis=0))
                s2 = sb.tile([P, F_HID], f32, tag="s2")
                nc.vector.reduce_sum(out=s2[:], in_=gt[:].rearrange("p k f -> p f k"),
                                     axis=mybir.AxisListType.X)
                sf = sb.tile([P, F_HID], f32, tag="sf")
                nc.sync.dma_start(sf[:], slab2[g * P:(g + 1) * P, :])
                nc.vector.tensor_add(s2[:], s2[:], sf[:])
                nc.vector.tensor_scalar_mul(s2[:], s2[:], dinv_s[:, g:g + 1])
                tpp = pp.tile([F_HID, P], f32, tag="tr")
                nc.tensor.transpose(tpp[:], s2[:], ident[:])
                s2T = sb.tile([F_HID, P], f32, tag="s2T")
                nc.vector.tensor_copy(s2T[:], tpp[:])
                op = pp.tile([P, F_OUT], f32, tag="mm2")
                nc.tensor.matmul(op[:], lhsT=s2T[:], rhs=W2_s[:],
                                 start=True, stop=True)
                of = sb.tile([P, F_OUT], f32, tag="of")
                nc.vector.tensor_add(of[:], op[:], b2r_s[:])
                nc.sync.dma_start(out_d[g * P:(g + 1) * P, :], of[:])

    nc.compile()
    return nc


def _assemble(results, meta):
    M = N_CORES
    Ns, N, F_OUT = meta["Ns"], meta["N"], meta["F_OUT"]
    out = np.empty((N, F_OUT), dtype=np.float32)
    for m in range(M):
        pos_of = meta["pos_of_list"][m]
        out[m * Ns:(m + 1) * Ns] = results[m]["out"][pos_of[:Ns]]
    return out


_CACHE = {}


def kernel(x, edge_index, W1, b1, W2, b2):
    meta, in_maps = _preprocess(x, edge_index, W1, b1, W2, b2)
    key = (meta["N"], meta["SUMK"], tuple(meta["Ks"]))
    if key not in _CACHE:
        _CACHE[key] = _build_program(meta)
    nc = _CACHE[key]
    from concourse import bass_utils
    res = bass_utils.run_bass_kernel_spmd(nc, in_maps, core_ids=list(range(N_CORES)))
    return _assemble(res.results, meta)



# revision 10
# speedup vs baseline: 1.0027x; 1.0027x over previous
"""Trainium2 Bass kernel for 2-layer GCN (nn_BasicGNN), v4.

Aggregation via InstDMAGatherAnt (one instruction per ~80-slot chunk,
vs one indirect DMA per 128 rows in the baseline):
  - Tables are bf16, node-major; gather elem = 256B = 8 consecutive rows.
    Each edge slot fetches the 8-row block holding its source; a host-built
    one-hot lane mask (bf16) times the block, then a strided reduce, yields
    the per-dest sums. Pads gather a zero block with a zero mask.
  - Layer 1 computed redundantly on every core into a local full table
    (no AllGather for layer 1); layer 2 table is dest-sharded + one
    AllGather.
  - Self-loop as an extra slot pointing at the node's own row.
  - Phase C: batched PE transposes + block-diagonal W2 matmuls.
"""

import sys
import numpy as np
from ml_dtypes import bfloat16

if "/opt/trn_rl_repo" not in sys.path:
    sys.path.insert(0, "/opt/trn_rl_repo")

N_CORES = 8
P = 128
N_NODES = 100000
F_IN = 128
F_HID = 16
F_OUT = 40

Ns = N_NODES // N_CORES            # 12500
NsP = ((Ns + P - 1) // P) * P      # 12544
G = NsP // P                       # 98
S2 = NsP + P                       # 12672 slab2 rows (last 128 zero)
NF = ((N_NODES + P - 1) // P) * P  # 100096
GF = NF // P                       # 782
NFP = NF + P                       # tab1 rows incl zero block
TB1 = NFP // 8                     # tab1 8-row blocks
PAD1B = NF // 8                    # zero block id in tab1
TAB2_ROWS = N_CORES * S2           # 101376
TB2 = TAB2_ROWS // 8
PAD2B = NsP // 8                   # core0 zero rows start -> block
CH = 56                            # gather slots per chunk (<=7168 descs/DMA)


def _preprocess(x, edge_index, W1, b1, W2, b2):
    x = np.asarray(x, dtype=np.float32)
    W1 = np.asarray(W1, dtype=np.float32)
    b1 = np.asarray(b1, dtype=np.float32)
    W2 = np.asarray(W2, dtype=np.float32)
    b2 = np.asarray(b2, dtype=np.float32)
    N = x.shape[0]
    assert N == N_NODES and x.shape[1] == F_IN
    M = N_CORES

    row = np.asarray(edge_index[0]).astype(np.int64)
    col = np.asarray(edge_index[1]).astype(np.int64)
    deg = np.bincount(col, minlength=N).astype(np.int64) + 1
    dinv = (deg.astype(np.float64) ** -0.5).astype(np.float32)

    pos_of_list = []
    cnt_list = []
    for m in range(M):
        indeg = deg[m * Ns:(m + 1) * Ns] - 1
        key = np.concatenate([indeg, np.full(NsP - Ns, -1, dtype=np.int64)])
        order = np.argsort(key, kind="stable")
        pos_of = np.empty(NsP, dtype=np.int64)
        pos_of[order] = np.arange(NsP)
        pos_of_list.append(pos_of)
        cnt_list.append(np.maximum(key[order], 0))

    Ks = np.zeros(G, dtype=np.int64)
    for m in range(M):
        Ks = np.maximum(Ks, cnt_list[m].reshape(G, P).max(axis=1))
    Ks = Ks + 1
    offs = np.zeros(G + 1, dtype=np.int64)
    offs[1:] = np.cumsum(Ks)
    SUMK = int(offs[-1])

    u = np.arange(N, dtype=np.int64)
    pos_all = np.empty(N, dtype=np.int64)
    for m in range(M):
        pos_all[m * Ns:(m + 1) * Ns] = pos_of_list[m][:Ns]
    r2_all = (u // Ns) * S2 + pos_all

    xd = x * dinv[:, None]
    xdp = np.zeros((NF, F_IN), np.float32)
    xdp[:N] = xd
    xT = np.ascontiguousarray(xdp.T).astype(bfloat16)

    W1b = W1.astype(bfloat16)
    W2blk = np.zeros((P, 8 * F_OUT), np.float32)
    for g in range(8):
        W2blk[g * F_HID:(g + 1) * F_HID, g * F_OUT:(g + 1) * F_OUT] = W2
    b2blk = np.tile(b2[None, :], (P, 8)).astype(np.float32)

    def wrap_idx(flat):
        # flat [128*SUMK] int16 -> [128, 8*SUMK] (16-partition wrap, x8)
        w16 = flat.reshape(8 * SUMK, 16).T
        return np.ascontiguousarray(np.tile(w16, (8, 1)))

    in_maps = []
    for m in range(M):
        pos_of = pos_of_list[m]
        mask = (col >= m * Ns) & (col < (m + 1) * Ns)
        er = row[mask]
        dpos = pos_of[col[mask] - m * Ns]
        o = np.argsort(dpos, kind="stable")
        dpos = dpos[o]
        er = er[o]
        cnt_nz = np.bincount(dpos, minlength=NsP)
        starts = np.concatenate([[0], np.cumsum(cnt_nz)])[:-1]
        rank = np.arange(len(dpos)) - starts[dpos]
        g_of = dpos // P
        p_of = dpos % P
        slot = offs[g_of] + rank          # column kk in [0, SUMK)
        d = np.arange(Ns, dtype=np.int64)
        pos_d = pos_of[:Ns]
        sslot = offs[pos_d // P] + cnt_nz[pos_d]
        sp = pos_d % P

        # layer-1: node-major table, block u//8 lane u%8
        blkB = np.full((P, SUMK), PAD1B, dtype=np.int16)
        lanB = np.zeros((P, SUMK), dtype=np.int8)
        mskB = np.zeros((P, SUMK), dtype=bool)
        blkB[p_of, slot] = (er // 8).astype(np.int16)
        lanB[p_of, slot] = (er % 8).astype(np.int8)
        mskB[p_of, slot] = True
        own = m * Ns + d
        blkB[sp, sslot] = (own // 8).astype(np.int16)
        lanB[sp, sslot] = (own % 8).astype(np.int8)
        mskB[sp, sslot] = True

        # layer-2: tab2 row r2, block r2//8 lane r2%8
        r2e = r2_all[er]
        blkC = np.full((P, SUMK), PAD2B, dtype=np.int16)
        lanC = np.zeros((P, SUMK), dtype=np.int8)
        mskC = np.zeros((P, SUMK), dtype=bool)
        blkC[p_of, slot] = (r2e // 8).astype(np.int16)
        lanC[p_of, slot] = (r2e % 8).astype(np.int8)
        mskC[p_of, slot] = True
        r2o = r2_all[own]
        blkC[sp, sslot] = (r2o // 8).astype(np.int16)
        lanC[sp, sslot] = (r2o % 8).astype(np.int8)
        mskC[sp, sslot] = True

        # flat gather order: slot-major, i = kk*128 + p
        idxB_w = wrap_idx(np.ascontiguousarray(blkB.T).reshape(-1))
        idxC_w = wrap_idx(np.ascontiguousarray(blkC.T).reshape(-1))

        # masks [128, SUMK*8] bf16 one-hot over lanes
        def onehot(lan, msk):
            oh = np.zeros((P, SUMK, 8), np.float32)
            pp, kk = np.nonzero(msk)
            oh[pp, kk, lan[pp, kk].astype(np.int64)] = 1.0
            return np.ascontiguousarray(oh.reshape(P, SUMK * 8)).astype(bfloat16)

        maskB = onehot(lanB, mskB)
        maskC = onehot(lanC, mskC)

        d_sorted = np.zeros(NsP, np.float32)
        d_sorted[pos_d] = dinv[m * Ns:(m + 1) * Ns]
        de = d_sorted.reshape(G, P).T
        dinv2e = np.ascontiguousarray(np.repeat(de * de, F_HID, axis=1))
        dinvCe = np.ascontiguousarray(np.repeat(de, F_HID, axis=1))
        db1e = np.ascontiguousarray(
            np.repeat(de, F_HID, axis=1) * np.tile(b1, (P, G)))

        in_maps.append({
            "xT": xT,
            "idxB": idxB_w,
            "idxC": idxC_w,
            "maskB": maskB,
            "maskC": maskC,
            "dinv2e": dinv2e,
            "dinvCe": dinvCe,
            "db1e": db1e,
            "W1b": W1b,
            "W2blk": W2blk,
            "b2blk": b2blk,
        })

    meta = dict(N=N, SUMK=SUMK, Ks=Ks.tolist(), offs=offs.tolist(),
                pos_of_list=pos_of_list, b1_zero=bool(not np.any(b1)))
    return meta, in_maps


def _chunks(meta):
    Ks = meta["Ks"]
    budget = max(CH, max(Ks))
    out = []
    cur = []
    curk = 0
    for g in range(G):
        if cur and curk + Ks[g] > budget:
            out.append(cur)
            cur = []
            curk = 0
        cur.append(g)
        curk += Ks[g]
    if cur:
        out.append(cur)
    return out


def _build_program(meta):
    import concourse.bacc as bacc
    import concourse.tile as tile
    import concourse.mybir as mybir
    from concourse.masks import make_identity

    f32 = mybir.dt.float32
    bf16 = mybir.dt.bfloat16
    i16 = mybir.dt.int16
    SUMK = meta["SUMK"]
    offs = meta["offs"]
    M = N_CORES

    nc = bacc.Bacc("TRN2", target_bir_lowering=False, debug=False,
                   enable_asserts=False, num_devices=M)

    xT_d = nc.dram_tensor("xT", [P, NF], bf16, kind="ExternalInput")
    idxB_d = nc.dram_tensor("idxB", [P, 8 * SUMK], i16, kind="ExternalInput")
    idxC_d = nc.dram_tensor("idxC", [P, 8 * SUMK], i16, kind="ExternalInput")
    maskB_d = nc.dram_tensor("maskB", [P, 8 * SUMK], bf16, kind="ExternalInput")
    maskC_d = nc.dram_tensor("maskC", [P, 8 * SUMK], bf16, kind="ExternalInput")
    dinv2e_d = nc.dram_tensor("dinv2e", [P, G * F_HID], f32, kind="ExternalInput")
    dinvCe_d = nc.dram_tensor("dinvCe", [P, G * F_HID], f32, kind="ExternalInput")
    db1e_d = nc.dram_tensor("db1e", [P, G * F_HID], f32, kind="ExternalInput")
    W1b_d = nc.dram_tensor("W1b", [F_IN, F_HID], bf16, kind="ExternalInput")
    W2blk_d = nc.dram_tensor("W2blk", [P, 8 * F_OUT], f32, kind="ExternalInput")
    b2blk_d = nc.dram_tensor("b2blk", [P, 8 * F_OUT], f32, kind="ExternalInput")
    out_d = nc.dram_tensor("out", [P, G * F_OUT], f32, kind="ExternalOutput")

    tab1_d = nc.dram_tensor("tab1", [NFP, F_HID], bf16, kind="Internal")
    slab2_d = nc.dram_tensor("slab2", [S2, F_HID], bf16, kind="Internal")
    tab2_d = nc.dram_tensor("tab2", [TAB2_ROWS, F_HID], bf16, kind="Internal",
                            addr_space="Shared")
    RG = [list(range(M))]
    chunks = _chunks(meta)
    CHv = max(CH, max(meta["Ks"]))
    assert CHv * P <= 8192, f"chunk too big: {CHv}" 

    with tile.TileContext(nc) as tc:
        with tc.tile_pool(name="wts", bufs=1) as wp, \
             tc.tile_pool(name="xchunk", bufs=2) as xp, \
             tc.tile_pool(name="stage", bufs=3) as stp, \
             tc.tile_pool(name="gath", bufs=2) as gp, \
             tc.tile_pool(name="tmpp", bufs=2) as tpo, \
             tc.tile_pool(name="idxp", bufs=2) as ip, \
             tc.tile_pool(name="work", bufs=2) as sb, \
             tc.tile_pool(name="psA", bufs=2, space="PSUM") as pp, \
             tc.tile_pool(name="psT", bufs=2, space="PSUM") as tp, \
             tc.tile_pool(name="psM", bufs=2, space="PSUM") as mp:

            dinv2e_s = wp.tile([P, G * F_HID], f32)
            nc.sync.dma_start(dinv2e_s[:], dinv2e_d[:])
            dinvCe_s = wp.tile([P, G * F_HID], f32)
            nc.sync.dma_start(dinvCe_s[:], dinvCe_d[:])
            if not meta["b1_zero"]:
                db1e_s = wp.tile([P, G * F_HID], f32)
                nc.sync.dma_start(db1e_s[:], db1e_d[:])
            W1b_s = wp.tile([F_IN, F_HID], bf16)
            nc.sync.dma_start(W1b_s[:], W1b_d[:])
            W2blk_s = wp.tile([P, 8 * F_OUT], f32)
            nc.sync.dma_start(W2blk_s[:], W2blk_d[:])
            b2blk_s = wp.tile([P, 8 * F_OUT], f32)
            nc.sync.dma_start(b2blk_s[:], b2blk_d[:])
            ident = wp.tile([P, P], f32)
            make_identity(nc, ident[:])
            zt = wp.tile([P, F_HID], bf16)
            nc.vector.memset(zt[:], 0.0)
            nc.sync.dma_start(tab1_d[NF:NF + P, :], zt[:])
            nc.sync.dma_start(slab2_d[NsP:S2, :], zt[:])

            s1_big = wp.tile([P, G * F_HID], f32)
            gs2_big = wp.tile([P, G * F_HID], bf16)
            s2_big = wp.tile([P, G * F_HID], f32)
            out_big = wp.tile([P, G * F_OUT], f32)

            # ---- Phase A: full layer-1 table, redundant per core ----
            GXC = 64
            for c0 in range(0, GF, GXC):
                ng = min(GXC, GF - c0)
                xt = xp.tile([P, GXC * P], bf16, tag="xt")
                nc.sync.dma_start(xt[:, 0:ng * P],
                                  xT_d[:, c0 * P:(c0 + ng) * P])
                for b0 in range(0, ng, 6):
                    nb = min(6, ng - b0)
                    ps = pp.tile([P, 8 * F_HID], f32, tag="psA")
                    for j in range(nb):
                        nc.tensor.matmul(
                            ps[:, j * F_HID:(j + 1) * F_HID],
                            lhsT=xt[:, (b0 + j) * P:(b0 + j + 1) * P],
                            rhs=W1b_s[:], start=True, stop=True)
                    st = stp.tile([P, 8 * F_HID], bf16, tag="stA")
                    nc.vector.tensor_copy(st[:, 0:nb * F_HID],
                                          ps[:, 0:nb * F_HID])
                    gg = c0 + b0
                    nc.sync.dma_start(
                        tab1_d[gg * P:(gg + nb) * P, :].rearrange(
                            "(g p) f -> p g f", p=P),
                        st[:, 0:nb * F_HID])

            # ---- gather + mask + reduce for one layer ----
            def agg_layer(idx_d, mask_d, tab_view, s_out):
                for chunk in chunks:
                    o0, o1 = offs[chunk[0]], offs[chunk[-1] + 1]
                    ck = o1 - o0
                    ni = P * ck
                    it = ip.tile([P, 8 * CHv], i16, tag="idx")
                    nc.sync.dma_start(it[:, 0:8 * ck],
                                      idx_d[:, 8 * o0:8 * o1])
                    mt = ip.tile([P, 8 * CHv], bf16, tag="msk")
                    nc.sync.dma_start(mt[:, 0:8 * ck],
                                      mask_d[:, 8 * o0:8 * o1])
                    gt = gp.tile([P, CHv, 8 * F_HID], bf16, tag="gt")
                    nc.gpsimd.dma_gather(
                        out_ap=gt[:, 0:ck, :], in_ap=tab_view,
                        idxs_ap=it[:, 0:8 * ck], num_idxs=ni,
                        num_idxs_reg=ni, elem_size=8 * F_HID, single_packet=False)
                    tmp = tpo.tile([P, CHv, 8, F_HID], bf16, tag="tmp")
                    nc.vector.tensor_tensor(
                        out=tmp[:, 0:ck, :, :],
                        in0=gt[:, 0:ck, :].rearrange(
                            "p k (c f) -> p k c f", c=8),
                        in1=it_mask_bcast(mt, ck),
                        op=mybir.AluOpType.mult)
                    for g in chunk:
                        a = offs[g] - o0
                        b = offs[g + 1] - o0
                        nc.vector.reduce_sum(
                            out=s_out[:, g * F_HID:(g + 1) * F_HID],
                            in_=tmp[:, a:b, :, :].rearrange(
                                "p k c f -> p f k c"),
                            axis=mybir.AxisListType.XY)

            def it_mask_bcast(mt, ck):
                return mt[:, 0:8 * ck].rearrange(
                    "p (k c) -> p k c", c=8).to_broadcast(
                    [P, ck, 8, F_HID])

            # ---- Phase B ----
            tab1_view = tab1_d[:].rearrange("(b q) f -> b (q f)", q=8)
            agg_layer(idxB_d, maskB_d, tab1_view, s1_big)

            cg = G * F_HID
            nc.vector.tensor_mul(gs2_big[:], s1_big[:], dinv2e_s[:])
            if not meta["b1_zero"]:
                nc.vector.tensor_add(gs2_big[:], gs2_big[:], db1e_s[:])
            nc.vector.tensor_scalar_max(gs2_big[:], gs2_big[:], 0.0)
            for w0 in range(0, G, 6):
                wn = min(6, G - w0)
                nc.sync.dma_start(
                    slab2_d[w0 * P:(w0 + wn) * P, :].rearrange(
                        "(g p) f -> p g f", p=P),
                    gs2_big[:, w0 * F_HID:(w0 + wn) * F_HID])

            nc.gpsimd.collective_compute(
                "AllGather", mybir.AluOpType.bypass, replica_groups=RG,
                ins=[slab2_d[:]], outs=[tab2_d[:]])

            # ---- Phase C ----
            tab2_view = tab2_d[:].rearrange("(b q) f -> b (q f)", q=8)
            agg_layer(idxC_d, maskC_d, tab2_view, s2_big)

            nc.vector.tensor_mul(s2_big[:], s2_big[:], dinvCe_s[:])
            for b0 in range(0, G, 8):
                nb = min(8, G - b0)
                tpp = tp.tile([P, P], f32, tag="tr")
                nc.tensor.transpose(
                    tpp[0:nb * F_HID, :],
                    s2_big[:, b0 * F_HID:(b0 + nb) * F_HID], ident[:])
                bt = sb.tile([P, P], f32, tag="bt")
                nc.vector.tensor_copy(bt[0:nb * F_HID, :], tpp[0:nb * F_HID, :])
                mm = mp.tile([P, 8 * F_OUT], f32, tag="mm2")
                nc.tensor.matmul(
                    mm[:, 0:nb * F_OUT], lhsT=bt[0:nb * F_HID, :],
                    rhs=W2blk_s[0:nb * F_HID, 0:nb * F_OUT],
                    start=True, stop=True)
                nc.vector.tensor_add(
                    out_big[:, b0 * F_OUT:(b0 + nb) * F_OUT],
                    mm[:, 0:nb * F_OUT], b2blk_s[:, 0:nb * F_OUT])
            nc.sync.dma_start(out_d[:], out_big[:])

    nc.compile()
    return nc


def _assemble(results, meta):
    M = N_CORES
    N = meta["N"]
    out = np.empty((N, F_OUT), dtype=np.float32)
    for m in range(M):
        pos_of = meta["pos_of_list"][m]
        arr = results[m]["out"].reshape(P, G, F_OUT).transpose(1, 0, 2)
        arr = arr.reshape(NsP, F_OUT)
        out[m * Ns:(m + 1) * Ns] = arr[pos_of[:Ns]]
    return out


_CACHE = {}


def kernel(x, edge_index, W1, b1, W2, b2):
    meta, in_maps = _preprocess(x, edge_index, W1, b1, W2, b2)
    key = (meta["N"], meta["SUMK"], tuple(meta["Ks"]))
    if key not in _CACHE:
        _CACHE[key] = _build_program(meta)
    nc = _CACHE[key]
    from concourse import bass_utils
    res = bass_utils.run_bass_kernel_spmd(nc, in_maps, core_ids=list(range(N_CORES)))
    return _assemble(res.results, meta)
